# revision 13
# baseline (speedup 1.0000x reference)
"""DSGIAT GraphBranch kernel for trn2: full-device implementation on 8 cores.

Pipeline (all on device, single launch):
  GEMM1 (row-sharded, attention logits folded as extra columns) -> AllGather
  -> GAT agg (dma_gather of src rows + one-hot selector matmuls; softmax
     without max-subtraction) -> AG -> LP x2 (gather + selector matmul) with
     AG between -> GEMM2 -> AG -> GAT2 -> AG -> LP x2 -> transposed pooling
     via one-hot batch matmul -> AllReduce -> replicated MLP (transposed).

Sharding: nodes row-sharded 8 ways (3840 rows/core of 30720 padded); each
sparse pass processes edges whose dst is in the core's slab; exchanges via
ncfw AllGather. Payloads bf16, selectors fp8 ({0,1} exact), accum fp32.
"""
import os
import numpy as np
import ml_dtypes
from contextlib import ExitStack

BF16 = ml_dtypes.bfloat16
F8 = ml_dtypes.float8_e4m3

# ---- sizes (full problem; test_sim overrides via set_config) ----
CFG = dict(
    N=30000, NPAD=30720, IN_CH=256, OUT1=512, HID=128, HEADS=4,
    N_GRAPHS=64, NCORE=8,
)

NEG_SLOPE = 0.2
EPS = 1e-16
PAD = None  # set from cfg: pad gather index (a real, all-zero row)

_cached = {}


def set_config(**kw):
    CFG.update(kw)
    _cached.clear()


# ---------------- host preprocessing ----------------

def _wrap_idx(idx2d):
    """[T, EP] int -> dma_gather layout [128, T*(EP//16)] int16."""
    T, EP = idx2d.shape
    a = idx2d.reshape(T, EP // 16, 16).astype(np.int16)
    w16 = a.transpose(2, 0, 1)                      # [16, T, S]
    w = np.tile(w16, (8, 1, 1))                     # [128, T, S]
    return np.ascontiguousarray(w.reshape(128, -1))


def _edge_tiles(src, dst, ntiles, pad_idx, wgt=None):
    """Sort edges by dst, pad per dst-tile of 128. Returns
    (EP, src_pad [T,EP], dst_pad [T,EP], dstl [T,EP], w_pad or None)."""
    order = np.argsort(dst, kind="stable")
    s, d = src[order], dst[order]
    t = d // 128
    cnt = np.bincount(t, minlength=ntiles)
    EP = max(128, int(-(-cnt.max() // 128)) * 128)
    offs = np.zeros(ntiles, np.int64)
    offs[1:] = np.cumsum(cnt)[:-1]
    pos = np.arange(len(s)) - offs[t]
    sp = np.full((ntiles, EP), pad_idx, np.int32)
    dp = np.full((ntiles, EP), pad_idx, np.int32)
    dl = np.full((ntiles, EP), -1, np.int32)
    sp[t, pos] = s
    dp[t, pos] = d
    dl[t, pos] = d - t * 128
    wp = None
    if wgt is not None:
        wp = np.zeros((ntiles, EP), np.float32)
        wp[t, pos] = wgt[order]
    return EP, sp, dp, dl, wp


def _selector(dl):
    """One-hot [T, EP, 128] fp8 from dst-local indices (-1 -> zero row)."""
    T, EP = dl.shape
    S = np.zeros((T, EP, 128), np.uint8)
    ti, ei = np.nonzero(dl >= 0)
    S[ti, ei, dl[ti, ei]] = 1
    return S


def _sel_layout(S):
    """[T, EP, 128] -> [128, T*EP] fp8 (edge e=c*128+p of tile t at
    [p, t*EP + c*128 + n])."""
    T, EP, _ = S.shape
    CH = EP // 128
    out = S.reshape(T, CH, 128, 128).transpose(2, 0, 1, 3).reshape(128, -1)
    return out.astype(F8)


def _chunk_layout(v):
    """[T, EP] -> [128, T*(EP//128)]: value of edge c*128+p at [p, t*CH+c]."""
    T, EP = v.shape
    CH = EP // 128
    return np.ascontiguousarray(
        v.reshape(T, CH, 128).transpose(2, 0, 1).reshape(128, -1))


def _fold_logit_w(W, a_src, a_dst, heads, hid):
    ws = np.stack([W[:, h * hid:(h + 1) * hid] @ a_src[h] for h in range(heads)],
                  axis=1)
    wd = np.stack([W[:, h * hid:(h + 1) * hid] @ a_dst[h] for h in range(heads)],
                  axis=1)
    return ws, wd  # [in, heads]


# ---------------- device program ----------------

def _build(key):
    import concourse.tile as tile
    from concourse import bacc, mybir, bass

    EPG, EPL, NCORE = key
    c = CFG
    NPAD, IN_CH, OUT1, HEADS = c["NPAD"], c["IN_CH"], c["OUT1"], c["HEADS"]
    NG = c["N_GRAPHS"]
    NT = NPAD // 128
    TPC = NT // NCORE
    SLAB = TPC * 128
    KIN = IN_CH // 128          # k-chunks for GEMM1
    KH = OUT1 // 128            # k-chunks for GEMM2 / feature blocks
    CHG = EPG // 128
    CHL = EPL // 128
    GS = EPG // 16              # idx slots per tile (gather1)
    LS = EPL // 16
    W1C = OUT1 + 2 * HEADS      # 520
    STR = ((W1C * 2 + 255) // 256) * 128  # row stride elems (640) bf16
    JK = IN_CH + 2 * OUT1
    FCH = JK // 128             # 10 pooled feature chunks
    MLP1 = 256
    OC = MLP1 // 128            # 2

    bf = mybir.dt.bfloat16
    f32 = mybir.dt.float32
    f8 = mybir.dt.float8e4
    i16 = mybir.dt.int16

    nc = bacc.Bacc("TRN2", target_bir_lowering=False, debug=False,
                   num_devices=NCORE)

    # ---- inputs ----
    xts_d = nc.dram_tensor("xts", [IN_CH, SLAB], bf, kind="ExternalInput")
    xr_d = nc.dram_tensor("xr", [SLAB, IN_CH], bf, kind="ExternalInput")
    w1_d = nc.dram_tensor("w1e", [IN_CH, W1C], bf, kind="ExternalInput")
    w2_d = nc.dram_tensor("w2e", [OUT1, W1C], bf, kind="ExternalInput")
    b1_d = nc.dram_tensor("b1r", [128, OUT1], bf, kind="ExternalInput")
    b2_d = nc.dram_tensor("b2r", [128, OUT1], bf, kind="ExternalInput")
    gidx_d = nc.dram_tensor("gidx", [128, TPC * GS], i16, kind="ExternalInput")
    geidx_d = nc.dram_tensor("geidx", [128, TPC * 2 * GS], i16,
                             kind="ExternalInput")
    sgat_d = nc.dram_tensor("sgat", [128, TPC * EPG], f8, kind="ExternalInput")
    lidx_d = nc.dram_tensor("lidx", [128, TPC * LS], i16, kind="ExternalInput")
    slp_d = nc.dram_tensor("slp", [128, TPC * EPL], f8, kind="ExternalInput")
    wlp_d = nc.dram_tensor("wlp", [128, TPC * CHL], bf, kind="ExternalInput")
    spool_d = nc.dram_tensor("spool", [128, TPC * NG], f8, kind="ExternalInput")
    mw1_d = nc.dram_tensor("mw1", [128, FCH * OC * 128], bf,
                           kind="ExternalInput")
    mw2_d = nc.dram_tensor("mw2", [128, OC * 128], bf, kind="ExternalInput")
    mb1_d = nc.dram_tensor("mb1", [128, OC], f32, kind="ExternalInput")
    mb2_d = nc.dram_tensor("mb2", [128, 1], f32, kind="ExternalInput")
    rcnt_d = nc.dram_tensor("rcnt", [128, NG], f32, kind="ExternalInput")
    outT_d = nc.dram_tensor("outT", [128, NG], f32, kind="ExternalOutput")

    # ---- internal DRAM ----
    def idram(name, shape, dt, shared=False):
        return nc.dram_tensor(name, shape, dt, kind="Internal",
                              addr_space="Shared" if shared else "Local")

    sh = NCORE > 4
    h1p_own = idram("h1p_own", [SLAB, STR], bf)
    h1p = idram("h1p_full", [NPAD, STR], bf, shared=sh)
    h10_own = idram("h10_own", [SLAB, OUT1], f8)
    h10 = idram("h10_full", [NPAD, OUT1], f8, shared=sh)
    h1a_own = idram("h1a_own", [SLAB, OUT1], f8)
    h1a = idram("h1a_full", [NPAD, OUT1], f8, shared=sh)
    h1f_own = idram("h1f_own", [SLAB, OUT1], bf)
    h2p_own = idram("h2p_own", [SLAB, STR], bf)
    h2p = idram("h2p_full", [NPAD, STR], bf, shared=sh)
    h20_own = idram("h20_own", [SLAB, OUT1], f8)
    h20 = idram("h20_full", [NPAD, OUT1], f8, shared=sh)
    h2a_own = idram("h2a_own", [SLAB, OUT1], f8)
    h2a = idram("h2a_full", [NPAD, OUT1], f8, shared=sh)
    h2f_own = idram("h2f_own", [SLAB, OUT1], bf)
    ar_in = idram("ar_in", [128, FCH * NG], f32)
    ar_out = idram("ar_out", [128, FCH * NG], f32, shared=sh)

    ts = bass.ts
    RG = [list(range(NCORE))]
    AF = mybir.ActivationFunctionType
    ALU = mybir.AluOpType

    with tile.TileContext(nc) as tc, ExitStack() as ctx:
        cst = ctx.enter_context(tc.tile_pool(name="cst", bufs=1))
        big = ctx.enter_context(tc.tile_pool(name="big", bufs=1))
        sb = ctx.enter_context(tc.tile_pool(name="sb", bufs=2))
        ps = ctx.enter_context(tc.tile_pool(name="ps", bufs=2, space="PSUM"))
        pp = ctx.enter_context(tc.tile_pool(name="pp", bufs=2, space="PSUM"))

        # resident constants
        xts = big.tile([128, KIN, SLAB], bf, tag="glhs")
        for k in range(KIN):
            nc.sync.dma_start(xts[:, k, :], xts_d[ts(k, 128), :])
        w1 = cst.tile([128, KIN, W1C], bf)
        for k in range(KIN):
            nc.sync.dma_start(w1[:, k, :], w1_d[ts(k, 128), :])
        w2 = cst.tile([128, KH, W1C], bf)
        for k in range(KH):
            nc.sync.dma_start(w2[:, k, :], w2_d[ts(k, 128), :])
        b1r = cst.tile([128, OUT1], bf)
        nc.sync.dma_start(b1r[:], b1_d[:])
        b2r = cst.tile([128, OUT1], bf)
        nc.sync.dma_start(b2r[:], b2_d[:])
        gidx = cst.tile([128, TPC * GS], i16)
        nc.sync.dma_start(gidx[:], gidx_d[:])
        geidx = cst.tile([128, TPC * 2 * GS], i16)
        nc.sync.dma_start(geidx[:], geidx_d[:])
        lidx = cst.tile([128, TPC * LS], i16)
        nc.sync.dma_start(lidx[:], lidx_d[:])
        wlp = cst.tile([128, TPC * CHL], bf)
        nc.sync.dma_start(wlp[:], wlp_d[:])
        spool = cst.tile([128, TPC * NG], f8)
        nc.sync.dma_start(spool[:], spool_d[:])
        rcnt = cst.tile([128, NG], f32)
        nc.sync.dma_start(rcnt[:], rcnt_d[:])
        mb1 = cst.tile([128, OC], f32)
        nc.sync.dma_start(mb1[:], mb1_d[:])
        mb2 = cst.tile([128, 1], f32)
        nc.sync.dma_start(mb2[:], mb2_d[:])

        def gemm(wt, kch, src_get, dst):
            """dst[t rows] = lhsT_chunks^T @ wt ([128,kch,W1C])."""
            for t in range(TPC):
                pa = ps.tile([128, OUT1], f32, tag="acc")
                pb = ps.tile([128, 2 * HEADS], f32, tag="acc2")
                for k in range(kch):
                    lh = src_get(k, t)
                    nc.tensor.matmul(pa[:], lhsT=lh, rhs=wt[:, k, 0:OUT1],
                                     start=(k == 0), stop=(k == kch - 1))
                    nc.tensor.matmul(pb[:], lhsT=lh, rhs=wt[:, k, OUT1:W1C],
                                     start=(k == 0), stop=(k == kch - 1))
                ot = sb.tile([128, STR], bf, tag="geo")
                nc.vector.memset(ot[:, W1C:STR], 0.0)
                nc.vector.tensor_copy(ot[:, 0:OUT1], pa[:])
                nc.vector.tensor_copy(ot[:, OUT1:W1C], pb[:])
                nc.sync.dma_start(dst[ts(t, 128), :], ot[:])

        # ---- GEMM1 ----
        gemm(w1, KIN, lambda k, t: xts[:, k, ts(t, 128)], h1p_own)
        nc.gpsimd.collective_compute("AllGather", ALU.bypass,
                                     replica_groups=RG,
                                     ins=[h1p_own[:]], outs=[h1p[:]])

        def gat_pass(hp_full, out_own, out_bounce):
            for t in range(TPC):
                gh = sb.tile([128, CHG, OUT1], bf, tag="gbig")
                nc.gpsimd.dma_gather(
                    out_ap=gh[:], in_ap=hp_full[:, 0:OUT1],
                    idxs_ap=gidx[:, ts(t, GS)], num_idxs=EPG,
                    num_idxs_reg=EPG, elem_size=OUT1, elem_step=STR,
                    single_packet=False)
                ge = sb.tile([128, 2 * CHG, 128], bf, tag="ge")
                nc.gpsimd.dma_gather(
                    out_ap=ge[:], in_ap=hp_full[:, OUT1:STR],
                    idxs_ap=geidx[:, ts(t, 2 * GS)], num_idxs=2 * EPG,
                    num_idxs_reg=2 * EPG, elem_size=STR - OUT1,
                    elem_step=STR, single_packet=False)
                st = sb.tile([128, CHG, 128], f8, tag="sel")
                nc.sync.dma_start(
                    st[:].rearrange("p a b -> p (a b)"), sgat_d[:, ts(t, EPG)])
                lg = sb.tile([128, CHG, HEADS], f32, tag="lg")
                nc.vector.tensor_add(lg[:], ge[:, 0:CHG, 0:HEADS],
                                     ge[:, CHG:2 * CHG, HEADS:2 * HEADS])
                # leaky_relu: max(0.2*x, x) in one DVE op
                nc.vector.scalar_tensor_tensor(
                    lg[:], lg[:], NEG_SLOPE, lg[:], ALU.mult, ALU.max)
                ex = sb.tile([128, CHG, HEADS], bf, tag="ex")
                nc.scalar.activation(ex[:], lg[:], AF.Exp)
                ghv = gh[:].rearrange("p a (h q) -> p a h q", h=HEADS)
                nc.vector.tensor_mul(
                    ghv, ghv,
                    ex[:].unsqueeze(3).broadcast_to(
                        [128, CHG, HEADS, OUT1 // HEADS]))
                pnum = ps.tile([128, OUT1], f32, tag="acc")
                pden = ps.tile([128, HEADS], f32, tag="acc2")
                for ch in range(CHG):
                    nc.tensor.matmul(pnum[:], lhsT=st[:, ch, :],
                                     rhs=gh[:, ch, :],
                                     start=(ch == 0), stop=(ch == CHG - 1))
                    nc.tensor.matmul(pden[:], lhsT=st[:, ch, :],
                                     rhs=ex[:, ch, :],
                                     start=(ch == 0), stop=(ch == CHG - 1))
                de = sb.tile([128, HEADS], f32, tag="de")
                nc.vector.tensor_scalar_add(de[:], pden[:], EPS)
                nc.vector.reciprocal(de[:], de[:])
                tmp = sb.tile([128, HEADS, OUT1 // HEADS], f32, tag="tf32")
                nc.vector.tensor_mul(
                    tmp[:], pnum[:].rearrange("p (h q) -> p h q", h=HEADS),
                    de[:].unsqueeze(2).broadcast_to(
                        [128, HEADS, OUT1 // HEADS]))
                ob = sb.tile([128, OUT1], f8, tag="obuf")
                nc.vector.tensor_add(
                    ob[:], tmp[:].rearrange("p h q -> p (h q)"), b1r[:])
                nc.vector.tensor_scalar_max(ob[:], ob[:], 0.0)
                nc.sync.dma_start(out_own[ts(t, 128), :], ob[:])
                if out_bounce is not None:
                    nc.sync.dma_start(out_bounce[ts(t, 128), :], ob[:])

        def lp_pass(h_full, res_own, out_own, out_dt):
            for t in range(TPC):
                g = sb.tile([128, CHL, OUT1], f8, tag="gbig")
                nc.gpsimd.dma_gather(
                    out_ap=g[:], in_ap=h_full[:], idxs_ap=lidx[:, ts(t, LS)],
                    num_idxs=EPL, num_idxs_reg=EPL, elem_size=OUT1,
                    elem_step=OUT1, single_packet=False)
                st = sb.tile([128, CHL, 128], f8, tag="sel")
                nc.sync.dma_start(
                    st[:].rearrange("p a b -> p (a b)"), slp_d[:, ts(t, EPL)])
                msg = sb.tile([128, CHL, OUT1], bf, tag="lmsg")
                nc.vector.tensor_mul(
                    msg[:], g[:],
                    wlp[:, ts(t, CHL)].unsqueeze(2).broadcast_to(
                        [128, CHL, OUT1]))
                pa = ps.tile([128, OUT1], f32, tag="acc")
                for ch in range(CHL):
                    nc.tensor.matmul(pa[:], lhsT=st[:, ch, :],
                                     rhs=msg[:, ch, :],
                                     start=(ch == 0), stop=(ch == CHL - 1))
                res = sb.tile([128, OUT1], f8, tag="lres")
                nc.sync.dma_start(res[:], res_own[ts(t, 128), :])
                tf = sb.tile([128, OUT1], f32, tag="tf32")
                nc.vector.tensor_add(tf[:], pa[:], res[:])
                ob = sb.tile([128, OUT1], out_dt, tag="obuf")
                nc.vector.tensor_scalar(ob[:], tf[:], 0.5, 1.0,
                                        ALU.mult, ALU.min)
                nc.vector.tensor_scalar_max(ob[:], ob[:], 0.0)
                nc.sync.dma_start(out_own[ts(t, 128), :], ob[:])

        # ---- conv1: GAT + LP x2 ----
        gat_pass(h1p, h10_own, None)
        nc.gpsimd.collective_compute("AllGather", ALU.bypass,
                                     replica_groups=RG,
                                     ins=[h10_own[:]], outs=[h10[:]])
        lp_pass(h10, h10_own, h1a_own, f8)
        nc.gpsimd.collective_compute("AllGather", ALU.bypass,
                                     replica_groups=RG,
                                     ins=[h1a_own[:]], outs=[h1a[:]])
        lp_pass(h1a, h10_own, h1f_own, bf)

        # ---- GEMM2 (lhsT via DMA transpose of h1f_own) ----
        h1t = big.tile([128, KH, SLAB], bf, tag="glhs")
        for k in range(KH):
            nc.sync.dma_start(h1t[:, k, :], h1f_own[:, ts(k, 128)],
                              transpose=True)
        gemm(w2, KH, lambda k, t: h1t[:, k, ts(t, 128)], h2p_own)
        nc.gpsimd.collective_compute("AllGather", ALU.bypass,
                                     replica_groups=RG,
                                     ins=[h2p_own[:]], outs=[h2p[:]])

        # ---- conv2: GAT + LP x2 ----
        gat_pass(h2p, h20_own, None)
        nc.gpsimd.collective_compute("AllGather", ALU.bypass,
                                     replica_groups=RG,
                                     ins=[h20_own[:]], outs=[h20[:]])
        lp_pass(h20, h20_own, h2a_own, f8)
        nc.gpsimd.collective_compute("AllGather", ALU.bypass,
                                     replica_groups=RG,
                                     ins=[h2a_own[:]], outs=[h2a[:]])
        lp_pass(h2a, h20_own, h2f_own, bf)

        # ---- pooling (transposed): pooledT[f, g], one PSUM group per chunk --
        par = big.tile([128, FCH * NG], f32)
        psrc = ([(xr_d, k) for k in range(KIN)]
                + [(h1f_own, k) for k in range(KH)]
                + [(h2f_own, k) for k in range(KH)])
        for kk, (srcd, k) in enumerate(psrc):
            pk = pp.tile([128, NG], f32, tag="poolk")
            for t in range(TPC):
                lh = sb.tile([128, 128], bf, tag="plh")
                nc.sync.dma_start(lh[:], srcd[ts(t, 128), ts(k, 128)])
                nc.tensor.matmul(pk[:], lhsT=lh[:], rhs=spool[:, ts(t, NG)],
                                 start=(t == 0), stop=(t == TPC - 1))
            nc.vector.tensor_copy(par[:, ts(kk, NG)], pk[:])
        nc.sync.dma_start(ar_in[:], par[:])
        nc.gpsimd.collective_compute("AllReduce", ALU.add,
                                     replica_groups=RG,
                                     ins=[ar_in[:]], outs=[ar_out[:]])
        pool = big.tile([128, FCH, NG], f32)
        nc.sync.dma_start(pool[:].rearrange("p a b -> p (a b)"), ar_out[:])
        nc.vector.tensor_mul(
            pool[:], pool[:],
            rcnt[:].unsqueeze(1).broadcast_to([128, FCH, NG]))
        poolb = big.tile([128, FCH, NG], bf)
        nc.vector.tensor_copy(poolb[:], pool[:])

        # ---- MLP (transposed) ----
        mw1 = cst.tile([128, FCH, OC, 128], bf)
        nc.sync.dma_start(mw1[:].rearrange("p a b q -> p (a b q)"), mw1_d[:])
        mw2 = cst.tile([128, OC, 128], bf)
        nc.sync.dma_start(mw2[:].rearrange("p a b -> p (a b)"), mw2_d[:])
        hdd = big.tile([128, OC, NG], bf)
        for cc in range(OC):
            hps = pp.tile([128, NG], f32, tag="poolk")
            for k in range(FCH):
                nc.tensor.matmul(hps[:], lhsT=mw1[:, k, cc, :],
                                 rhs=poolb[:, k, :],
                                 start=(k == 0), stop=(k == FCH - 1))
            nc.vector.tensor_scalar(hdd[:, cc, :], hps[:],
                                    mb1[:, cc:cc + 1], 0.0, ALU.add, ALU.max)
        ops = pp.tile([128, NG], f32, tag="ops")
        for cc in range(OC):
            nc.tensor.matmul(ops[:], lhsT=mw2[:, cc, :], rhs=hdd[:, cc, :],
                             start=(cc == 0), stop=(cc == OC - 1))
        ofin = big.tile([128, NG], f32)
        nc.vector.tensor_scalar_add(ofin[:], ops[:], mb2[:, 0:1])
        nc.sync.dma_start(outT_d[:], ofin[:])

    nc.compile()
    return nc


# ---------------- host driver ----------------

def _prepare(x, edge_index, batch,
             conv1_W, conv1_asrc, conv1_adst, conv1_b,
             conv2_W, conv2_asrc, conv2_adst, conv2_b,
             mlp_W1, mlp_b1, mlp_W2, mlp_b2):
    c = CFG
    N, NPAD, IN_CH, OUT1, HEADS, HID = (c["N"], c["NPAD"], c["IN_CH"],
                                        c["OUT1"], c["HEADS"], c["HID"])
    NG, NCORE = c["N_GRAPHS"], c["NCORE"]
    NT = NPAD // 128
    TPC = NT // NCORE
    SLAB = TPC * 128
    pad_idx = N  # zero row

    src = np.asarray(edge_index[0], np.int64).astype(np.int32)
    dst = np.asarray(edge_index[1], np.int64).astype(np.int32)
    batch = np.asarray(batch, np.int64).astype(np.int32)
    loop = np.arange(N, dtype=np.int32)
    csrc = np.concatenate([src, loop])
    cdst = np.concatenate([dst, loop])

    deg = np.bincount(dst, minlength=N).astype(np.float32)
    dis = np.where(deg > 0, 1.0 / np.sqrt(np.maximum(deg, 1.0)), 0.0)
    wgt = dis[src] * dis[dst]

    EPG, gsp, gdp, gdl, _ = _edge_tiles(csrc, cdst, NT, pad_idx)
    EPL, lsp, _, ldl, lwp = _edge_tiles(src, dst, NT, pad_idx, wgt)
    Sg = _selector(gdl)
    Sl = _selector(ldl)

    # pooling one-hot
    bpad = np.full(NPAD, -1, np.int32)
    bpad[:N] = batch
    Spool = np.zeros((NT, 128, NG), np.uint8)
    ti = np.repeat(np.arange(NT), 128)
    pi = np.tile(np.arange(128), NT)
    v = bpad >= 0
    Spool[ti[v], pi[v], bpad[v]] = 1

    cnt = np.bincount(batch, minlength=NG).astype(np.float32)
    rcnt = (1.0 / np.maximum(cnt, 1.0)).astype(np.float32)

    x = np.asarray(x, np.float32)
    xp = np.zeros((NPAD, IN_CH), np.float32)
    xp[:N] = x
    xT = np.ascontiguousarray(xp.T).astype(BF16)
    xrows = xp.astype(BF16)

    def fold(W, asrc, adst):
        ws, wd = _fold_logit_w(np.asarray(W, np.float32),
                               np.asarray(asrc, np.float32),
                               np.asarray(adst, np.float32), HEADS, HID)
        return np.concatenate([np.asarray(W, np.float32), ws, wd],
                              axis=1).astype(BF16)

    w1e = fold(conv1_W, conv1_asrc, conv1_adst)
    w2e = fold(conv2_W, conv2_asrc, conv2_adst)
    b1r = np.tile(np.asarray(conv1_b, np.float32)[None, :],
                  (128, 1)).astype(BF16)
    b2r = np.tile(np.asarray(conv2_b, np.float32)[None, :],
                  (128, 1)).astype(BF16)

    W1 = np.asarray(mlp_W1, np.float32)     # [JK, 256]
    W2 = np.asarray(mlp_W2, np.float32)     # [256, 128]
    JK = W1.shape[0]
    FCH = JK // 128
    OC = W1.shape[1] // 128
    mw1 = np.ascontiguousarray(
        W1.reshape(FCH, 128, OC, 128).transpose(1, 0, 2, 3)
        .reshape(128, -1)).astype(BF16)
    mw2 = np.ascontiguousarray(
        W2.reshape(OC, 128, W2.shape[1]).transpose(1, 0, 2)
        .reshape(128, -1)).astype(BF16)
    mb1 = np.ascontiguousarray(
        np.asarray(mlp_b1, np.float32).reshape(OC, 128).T)
    mb2 = np.asarray(mlp_b2, np.float32).reshape(128, 1)
    rcT = np.tile(rcnt[None, :], (128, 1))

    in_maps = []
    for cix in range(NCORE):
        tl = slice(cix * TPC, (cix + 1) * TPC)
        rows = slice(cix * SLAB, (cix + 1) * SLAB)
        in_maps.append({
            "xts": np.ascontiguousarray(xT[:, rows]),
            "xr": np.ascontiguousarray(xrows[rows]),
            "w1e": w1e, "w2e": w2e, "b1r": b1r, "b2r": b2r,
            "gidx": _wrap_idx(gsp[tl]),
            "geidx": _wrap_idx(np.concatenate([gsp[tl], gdp[tl]], axis=1)),
            "sgat": _sel_layout(Sg[tl]),
            "lidx": _wrap_idx(lsp[tl]),
            "slp": _sel_layout(Sl[tl]),
            "wlp": _chunk_layout(lwp[tl]).astype(BF16),
            "spool": np.ascontiguousarray(
                Spool[tl].transpose(1, 0, 2).reshape(128, -1)).astype(F8),
            "mw1": mw1, "mw2": mw2, "mb1": mb1, "mb2": mb2, "rcnt": rcT,
        })
    return (EPG, EPL), in_maps


def kernel(x, edge_index, batch,
           conv1_W, conv1_asrc, conv1_adst, conv1_b,
           conv2_W, conv2_asrc, conv2_adst, conv2_b,
           mlp_W1, mlp_b1, mlp_W2, mlp_b2):
    NCORE = CFG["NCORE"]
    (EPG, EPL), in_maps = _prepare(
        x, edge_index, batch, conv1_W, conv1_asrc, conv1_adst, conv1_b,
        conv2_W, conv2_asrc, conv2_adst, conv2_b,
        mlp_W1, mlp_b1, mlp_W2, mlp_b2)
    key = (EPG, EPL, NCORE)
    if _cached.get("key") != key:
        _cached["nc"] = _build(key)
        _cached["key"] = key
    nc = _cached["nc"]

    if os.environ.get("BASS_KERNEL_SIM", "0") == "1":
        from concourse.bass_interp import MultiCoreSim
        nw = int(os.environ.get("BASS_SIM_WORKERS",
                                str(min(NCORE, os.cpu_count() or 1))))
        sim = MultiCoreSim(nc, num_cores=NCORE, require_finite=False,
                           num_workers=nw)
        for cix, cs in enumerate(sim.cores.values()):
            for k, v in in_maps[cix].items():
                cs.tensor(k)[:] = v
        sim.simulate()
        _cached["sim_time_ns"] = int(sim.global_time)
        outT = np.asarray(sim.cores[0].tensor("outT"))
    else:
        from concourse.bass_utils import run_bass_kernel_spmd
        import time
        t0 = time.time()
        res = run_bass_kernel_spmd(nc, in_maps, core_ids=list(range(NCORE)))
        _cached["device_wall_ns"] = int((time.time() - t0) * 1e9)
        _cached["last_result"] = res
        outT = res.results[0]["outT"]
    return np.ascontiguousarray(outT.T.astype(np.float32))


# revision 14
# speedup vs baseline: 2198.2104x; 2198.2104x over previous
"""DSGIAT GraphBranch kernel for trn2: full-device implementation on 8 cores.

Pipeline (all on device, single launch):
  GEMM1 (row-sharded, attention logits folded as extra columns) -> AllGather
  -> GAT agg (dma_gather of src rows + one-hot selector matmuls; softmax
     without max-subtraction) -> AG -> LP x2 (gather + selector matmul) with
     AG between -> GEMM2 -> AG -> GAT2 -> AG -> LP x2 -> transposed pooling
     via one-hot batch matmul -> AllReduce -> replicated MLP (transposed).

Sharding: nodes row-sharded 8 ways (3840 rows/core of 30720 padded); each
sparse pass processes edges whose dst is in the core's slab; exchanges via
ncfw AllGather. Payloads bf16, selectors fp8 ({0,1} exact), accum fp32.
"""
import os
import numpy as np
import ml_dtypes
from contextlib import ExitStack

BF16 = ml_dtypes.bfloat16
F8 = ml_dtypes.float8_e4m3

# ---- sizes (full problem; test_sim overrides via set_config) ----
CFG = dict(
    N=30000, NPAD=30720, IN_CH=256, OUT1=512, HID=128, HEADS=4,
    N_GRAPHS=64, NCORE=8,
)

NEG_SLOPE = 0.2
EPS = 1e-16
PAD = None  # set from cfg: pad gather index (a real, all-zero row)

_cached = {}


def set_config(**kw):
    CFG.update(kw)
    _cached.clear()


# ---------------- host preprocessing ----------------

def _wrap_idx(idx2d):
    """[T, EP] int -> dma_gather layout [128, T*(EP//16)] int16."""
    T, EP = idx2d.shape
    a = idx2d.reshape(T, EP // 16, 16).astype(np.int16)
    w16 = a.transpose(2, 0, 1)                      # [16, T, S]
    w = np.tile(w16, (8, 1, 1))                     # [128, T, S]
    return np.ascontiguousarray(w.reshape(128, -1))


def _edge_tiles(src, dst, ntiles, pad_idx, wgt=None):
    """Sort edges by dst, pad per dst-tile of 128. Returns
    (EP, src_pad [T,EP], dst_pad [T,EP], dstl [T,EP], w_pad or None)."""
    order = np.argsort(dst, kind="stable")
    s, d = src[order], dst[order]
    t = d // 128
    cnt = np.bincount(t, minlength=ntiles)
    EP = max(128, int(-(-cnt.max() // 128)) * 128)
    offs = np.zeros(ntiles, np.int64)
    offs[1:] = np.cumsum(cnt)[:-1]
    pos = np.arange(len(s)) - offs[t]
    sp = np.full((ntiles, EP), pad_idx, np.int32)
    dp = np.full((ntiles, EP), pad_idx, np.int32)
    dl = np.full((ntiles, EP), -1, np.int32)
    sp[t, pos] = s
    dp[t, pos] = d
    dl[t, pos] = d - t * 128
    wp = None
    if wgt is not None:
        wp = np.zeros((ntiles, EP), np.float32)
        wp[t, pos] = wgt[order]
    return EP, sp, dp, dl, wp


def _selector(dl):
    """One-hot [T, EP, 128] fp8 from dst-local indices (-1 -> zero row)."""
    T, EP = dl.shape
    S = np.zeros((T, EP, 128), np.uint8)
    ti, ei = np.nonzero(dl >= 0)
    S[ti, ei, dl[ti, ei]] = 1
    return S


def _sel_layout(S):
    """[T, EP, 128] -> [128, T*EP] fp8 (edge e=c*128+p of tile t at
    [p, t*EP + c*128 + n])."""
    T, EP, _ = S.shape
    CH = EP // 128
    out = S.reshape(T, CH, 128, 128).transpose(2, 0, 1, 3).reshape(128, -1)
    return out.astype(F8)


def _chunk_layout(v):
    """[T, EP] -> [128, T*(EP//128)]: value of edge c*128+p at [p, t*CH+c]."""
    T, EP = v.shape
    CH = EP // 128
    return np.ascontiguousarray(
        v.reshape(T, CH, 128).transpose(2, 0, 1).reshape(128, -1))


def _fold_logit_w(W, a_src, a_dst, heads, hid):
    ws = np.stack([W[:, h * hid:(h + 1) * hid] @ a_src[h] for h in range(heads)],
                  axis=1)
    wd = np.stack([W[:, h * hid:(h + 1) * hid] @ a_dst[h] for h in range(heads)],
                  axis=1)
    return ws, wd  # [in, heads]


# ---------------- device program ----------------

def _build(key):
    import concourse.tile as tile
    from concourse import bacc, mybir, bass

    EPG, EPL, NCORE = key
    c = CFG
    NPAD, IN_CH, OUT1, HEADS = c["NPAD"], c["IN_CH"], c["OUT1"], c["HEADS"]
    NG = c["N_GRAPHS"]
    NT = NPAD // 128
    TPC = NT // NCORE
    SLAB = TPC * 128
    KIN = IN_CH // 128          # k-chunks for GEMM1
    KH = OUT1 // 128            # k-chunks for GEMM2 / feature blocks
    CHG = EPG // 128
    CHL = EPL // 128
    GS = EPG // 16              # idx slots per tile (gather1)
    LS = EPL // 16
    W1C = OUT1 + 2 * HEADS      # 520
    STR = ((W1C * 2 + 255) // 256) * 128  # row stride elems (640) bf16
    JK = IN_CH + 2 * OUT1
    FCH = JK // 128             # 10 pooled feature chunks
    MLP1 = 256
    OC = MLP1 // 128            # 2

    bf = mybir.dt.bfloat16
    f32 = mybir.dt.float32
    f8 = mybir.dt.float8e4
    i16 = mybir.dt.int16

    nc = bacc.Bacc("TRN2", target_bir_lowering=False, debug=False,
                   num_devices=NCORE)

    # ---- inputs ----
    xts_d = nc.dram_tensor("xts", [IN_CH, SLAB], bf, kind="ExternalInput")
    xr_d = nc.dram_tensor("xr", [SLAB, IN_CH], bf, kind="ExternalInput")
    w1_d = nc.dram_tensor("w1e", [IN_CH, W1C], bf, kind="ExternalInput")
    w2_d = nc.dram_tensor("w2e", [OUT1, W1C], bf, kind="ExternalInput")
    b1_d = nc.dram_tensor("b1r", [128, OUT1], bf, kind="ExternalInput")
    b2_d = nc.dram_tensor("b2r", [128, OUT1], bf, kind="ExternalInput")
    gidx_d = nc.dram_tensor("gidx", [128, TPC * GS], i16, kind="ExternalInput")
    geidx_d = nc.dram_tensor("geidx", [128, TPC * 2 * GS], i16,
                             kind="ExternalInput")
    sgat_d = nc.dram_tensor("sgat", [128, TPC * EPG], f8, kind="ExternalInput")
    lidx_d = nc.dram_tensor("lidx", [128, TPC * LS], i16, kind="ExternalInput")
    slp_d = nc.dram_tensor("slp", [128, TPC * EPL], f8, kind="ExternalInput")
    wlp_d = nc.dram_tensor("wlp", [128, TPC * CHL], bf, kind="ExternalInput")
    spool_d = nc.dram_tensor("spool", [128, TPC * NG], f8, kind="ExternalInput")
    mw1_d = nc.dram_tensor("mw1", [128, FCH * OC * 128], bf,
                           kind="ExternalInput")
    mw2_d = nc.dram_tensor("mw2", [128, OC * 128], bf, kind="ExternalInput")
    mb1_d = nc.dram_tensor("mb1", [128, OC], f32, kind="ExternalInput")
    mb2_d = nc.dram_tensor("mb2", [128, 1], f32, kind="ExternalInput")
    rcnt_d = nc.dram_tensor("rcnt", [128, NG], f32, kind="ExternalInput")
    outT_d = nc.dram_tensor("outT", [128, NG], f32, kind="ExternalOutput")

    # ---- internal DRAM ----
    def idram(name, shape, dt, shared=False):
        return nc.dram_tensor(name, shape, dt, kind="Internal",
                              addr_space="Shared" if shared else "Local")

    sh = NCORE > 4
    h1p_own = idram("h1p_own", [SLAB, STR], bf)
    h1p = idram("h1p_full", [NPAD, STR], bf, shared=sh)
    h10_own = idram("h10_own", [SLAB, OUT1], f8)
    h10 = idram("h10_full", [NPAD, OUT1], f8, shared=sh)
    h1a_own = idram("h1a_own", [SLAB, OUT1], f8)
    h1a = idram("h1a_full", [NPAD, OUT1], f8, shared=sh)
    h1f_own = idram("h1f_own", [SLAB, OUT1], bf)
    h2p_own = idram("h2p_own", [SLAB, STR], bf)
    h2p = idram("h2p_full", [NPAD, STR], bf, shared=sh)
    h20_own = idram("h20_own", [SLAB, OUT1], f8)
    h20 = idram("h20_full", [NPAD, OUT1], f8, shared=sh)
    h2a_own = idram("h2a_own", [SLAB, OUT1], f8)
    h2a = idram("h2a_full", [NPAD, OUT1], f8, shared=sh)
    h2f_own = idram("h2f_own", [SLAB, OUT1], bf)
    ar_in = idram("ar_in", [128, FCH * NG], f32)
    ar_out = idram("ar_out", [128, FCH * NG], f32, shared=sh)

    ts = bass.ts
    RG = [list(range(NCORE))]
    AF = mybir.ActivationFunctionType
    ALU = mybir.AluOpType
    HALF = SLAB // 2
    HNP = NCORE * HALF

    def ag2(own, full):
        for h in range(2):
            nc.gpsimd.collective_compute(
                "AllGather", ALU.bypass, replica_groups=RG,
                ins=[own[h * HALF:(h + 1) * HALF, :]],
                outs=[full[h * HNP:(h + 1) * HNP, :]])

    with tile.TileContext(nc) as tc, ExitStack() as ctx:
        cst = ctx.enter_context(tc.tile_pool(name="cst", bufs=1))
        big = ctx.enter_context(tc.tile_pool(name="big", bufs=1))
        sb = ctx.enter_context(tc.tile_pool(name="sb", bufs=2))
        ps = ctx.enter_context(tc.tile_pool(name="ps", bufs=2, space="PSUM"))
        pp = ctx.enter_context(tc.tile_pool(name="pp", bufs=2, space="PSUM"))

        # resident constants
        xts = big.tile([128, KIN, SLAB], bf, tag="glhs")
        for k in range(KIN):
            nc.sync.dma_start(xts[:, k, :], xts_d[ts(k, 128), :])
        w1 = cst.tile([128, KIN, W1C], bf)
        for k in range(KIN):
            nc.sync.dma_start(w1[:, k, :], w1_d[ts(k, 128), :])
        w2 = cst.tile([128, KH, W1C], bf)
        for k in range(KH):
            nc.sync.dma_start(w2[:, k, :], w2_d[ts(k, 128), :])
        b1r = cst.tile([128, OUT1], bf)
        nc.sync.dma_start(b1r[:], b1_d[:])
        b2r = cst.tile([128, OUT1], bf)
        nc.sync.dma_start(b2r[:], b2_d[:])
        gidx = cst.tile([128, TPC * GS], i16)
        nc.sync.dma_start(gidx[:], gidx_d[:])
        geidx = cst.tile([128, TPC * 2 * GS], i16)
        nc.sync.dma_start(geidx[:], geidx_d[:])
        lidx = cst.tile([128, TPC * LS], i16)
        nc.sync.dma_start(lidx[:], lidx_d[:])
        wlp = cst.tile([128, TPC * CHL], bf)
        nc.sync.dma_start(wlp[:], wlp_d[:])
        spool = cst.tile([128, TPC * NG], f8)
        nc.sync.dma_start(spool[:], spool_d[:])
        rcnt = cst.tile([128, NG], f32)
        nc.sync.dma_start(rcnt[:], rcnt_d[:])
        mb1 = cst.tile([128, OC], f32)
        nc.sync.dma_start(mb1[:], mb1_d[:])
        mb2 = cst.tile([128, 1], f32)
        nc.sync.dma_start(mb2[:], mb2_d[:])

        def gemm(wt, kch, src_get, dst):
            """dst[t rows] = lhsT_chunks^T @ wt ([128,kch,W1C])."""
            for t in range(TPC):
                pa = ps.tile([128, OUT1], f32, tag="acc")
                pb = ps.tile([128, 2 * HEADS], f32, tag="acc2")
                for k in range(kch):
                    lh = src_get(k, t)
                    nc.tensor.matmul(pa[:], lhsT=lh, rhs=wt[:, k, 0:OUT1],
                                     start=(k == 0), stop=(k == kch - 1))
                    nc.tensor.matmul(pb[:], lhsT=lh, rhs=wt[:, k, OUT1:W1C],
                                     start=(k == 0), stop=(k == kch - 1))
                ot = sb.tile([128, STR], bf, tag="geo")
                nc.vector.memset(ot[:, W1C:STR], 0.0)
                nc.vector.tensor_copy(ot[:, 0:OUT1], pa[:])
                nc.vector.tensor_copy(ot[:, OUT1:W1C], pb[:])
                nc.sync.dma_start(dst[ts(t, 128), :], ot[:])

        # ---- GEMM1 ----
        gemm(w1, KIN, lambda k, t: xts[:, k, ts(t, 128)], h1p_own)
        ag2(h1p_own, h1p)

        def gat_pass(hp_full, out_own, out_bounce):
            for t in range(TPC):
                gh = sb.tile([128, CHG, OUT1], bf, tag="gbig")
                nc.gpsimd.dma_gather(
                    out_ap=gh[:], in_ap=hp_full[:, 0:OUT1],
                    idxs_ap=gidx[:, ts(t, GS)], num_idxs=EPG,
                    num_idxs_reg=EPG, elem_size=OUT1, elem_step=STR,
                    single_packet=False)
                ge = sb.tile([128, 2 * CHG, 128], bf, tag="ge")
                nc.gpsimd.dma_gather(
                    out_ap=ge[:], in_ap=hp_full[:, OUT1:STR],
                    idxs_ap=geidx[:, ts(t, 2 * GS)], num_idxs=2 * EPG,
                    num_idxs_reg=2 * EPG, elem_size=STR - OUT1,
                    elem_step=STR, single_packet=False)
                st = sb.tile([128, CHG, 128], f8, tag="sel")
                nc.sync.dma_start(
                    st[:].rearrange("p a b -> p (a b)"), sgat_d[:, ts(t, EPG)])
                lg = sb.tile([128, CHG, HEADS], f32, tag="lg")
                nc.vector.tensor_add(lg[:], ge[:, 0:CHG, 0:HEADS],
                                     ge[:, CHG:2 * CHG, HEADS:2 * HEADS])
                # leaky_relu: max(0.2*x, x) in one DVE op
                nc.vector.scalar_tensor_tensor(
                    lg[:], lg[:], NEG_SLOPE, lg[:], ALU.mult, ALU.max)
                ex = sb.tile([128, CHG, HEADS], bf, tag="ex")
                nc.scalar.activation(ex[:], lg[:], AF.Exp)
                ghv = gh[:].rearrange("p a (h q) -> p a h q", h=HEADS)
                nc.vector.tensor_mul(
                    ghv, ghv,
                    ex[:].unsqueeze(3).broadcast_to(
                        [128, CHG, HEADS, OUT1 // HEADS]))
                pnum = ps.tile([128, OUT1], f32, tag="acc")
                pden = ps.tile([128, HEADS], f32, tag="acc2")
                for ch in range(CHG):
                    nc.tensor.matmul(pnum[:], lhsT=st[:, ch, :],
                                     rhs=gh[:, ch, :],
                                     start=(ch == 0), stop=(ch == CHG - 1))
                    nc.tensor.matmul(pden[:], lhsT=st[:, ch, :],
                                     rhs=ex[:, ch, :],
                                     start=(ch == 0), stop=(ch == CHG - 1))
                de = sb.tile([128, HEADS], f32, tag="de")
                nc.vector.tensor_scalar_add(de[:], pden[:], EPS)
                nc.vector.reciprocal(de[:], de[:])
                tmp = sb.tile([128, HEADS, OUT1 // HEADS], f32, tag="tf32")
                nc.vector.tensor_mul(
                    tmp[:], pnum[:].rearrange("p (h q) -> p h q", h=HEADS),
                    de[:].unsqueeze(2).broadcast_to(
                        [128, HEADS, OUT1 // HEADS]))
                ob = sb.tile([128, OUT1], f8, tag="obuf")
                nc.vector.tensor_add(
                    ob[:], tmp[:].rearrange("p h q -> p (h q)"), b1r[:])
                nc.vector.tensor_scalar_max(ob[:], ob[:], 0.0)
                nc.sync.dma_start(out_own[ts(t, 128), :], ob[:])
                if out_bounce is not None:
                    nc.sync.dma_start(out_bounce[ts(t, 128), :], ob[:])

        def lp_pass(h_full, res_own, out_own, out_dt):
            for t in range(TPC):
                g = sb.tile([128, CHL, OUT1], f8, tag="gbig")
                nc.gpsimd.dma_gather(
                    out_ap=g[:], in_ap=h_full[:], idxs_ap=lidx[:, ts(t, LS)],
                    num_idxs=EPL, num_idxs_reg=EPL, elem_size=OUT1,
                    elem_step=OUT1, single_packet=False)
                st = sb.tile([128, CHL, 128], f8, tag="sel")
                nc.sync.dma_start(
                    st[:].rearrange("p a b -> p (a b)"), slp_d[:, ts(t, EPL)])
                msg = sb.tile([128, CHL, OUT1], bf, tag="lmsg")
                nc.vector.tensor_mul(
                    msg[:], g[:],
                    wlp[:, ts(t, CHL)].unsqueeze(2).broadcast_to(
                        [128, CHL, OUT1]))
                pa = ps.tile([128, OUT1], f32, tag="acc")
                for ch in range(CHL):
                    nc.tensor.matmul(pa[:], lhsT=st[:, ch, :],
                                     rhs=msg[:, ch, :],
                                     start=(ch == 0), stop=(ch == CHL - 1))
                res = sb.tile([128, OUT1], f8, tag="lres")
                nc.sync.dma_start(res[:], res_own[ts(t, 128), :])
                tf = sb.tile([128, OUT1], f32, tag="tf32")
                nc.vector.tensor_add(tf[:], pa[:], res[:])
                ob = sb.tile([128, OUT1], out_dt, tag="obuf")
                nc.vector.tensor_scalar(ob[:], tf[:], 0.5, 1.0,
                                        ALU.mult, ALU.min)
                nc.vector.tensor_scalar_max(ob[:], ob[:], 0.0)
                nc.sync.dma_start(out_own[ts(t, 128), :], ob[:])

        # ---- conv1: GAT + LP x2 ----
        gat_pass(h1p, h10_own, None)
        ag2(h10_own, h10)
        lp_pass(h10, h10_own, h1a_own, f8)
        ag2(h1a_own, h1a)
        lp_pass(h1a, h10_own, h1f_own, bf)

        # ---- GEMM2 (lhsT via DMA transpose of h1f_own) ----
        h1t = big.tile([128, KH, SLAB], bf, tag="glhs")
        for k in range(KH):
            nc.sync.dma_start(h1t[:, k, :], h1f_own[:, ts(k, 128)],
                              transpose=True)
        gemm(w2, KH, lambda k, t: h1t[:, k, ts(t, 128)], h2p_own)
        ag2(h2p_own, h2p)

        # ---- conv2: GAT + LP x2 ----
        gat_pass(h2p, h20_own, None)
        ag2(h20_own, h20)
        lp_pass(h20, h20_own, h2a_own, f8)
        ag2(h2a_own, h2a)
        lp_pass(h2a, h20_own, h2f_own, bf)

        # ---- pooling (transposed): pooledT[f, g], one PSUM group per chunk --
        par = big.tile([128, FCH * NG], f32)
        psrc = ([(xr_d, k) for k in range(KIN)]
                + [(h1f_own, k) for k in range(KH)]
                + [(h2f_own, k) for k in range(KH)])
        for kk, (srcd, k) in enumerate(psrc):
            pk = pp.tile([128, NG], f32, tag="poolk")
            for t in range(TPC):
                lh = sb.tile([128, 128], bf, tag="plh")
                nc.sync.dma_start(lh[:], srcd[ts(t, 128), ts(k, 128)])
                nc.tensor.matmul(pk[:], lhsT=lh[:], rhs=spool[:, ts(t, NG)],
                                 start=(t == 0), stop=(t == TPC - 1))
            nc.vector.tensor_copy(par[:, ts(kk, NG)], pk[:])
        nc.sync.dma_start(ar_in[:], par[:])
        nc.gpsimd.collective_compute("AllReduce", ALU.add,
                                     replica_groups=RG,
                                     ins=[ar_in[:]], outs=[ar_out[:]])
        pool = big.tile([128, FCH, NG], f32)
        nc.sync.dma_start(pool[:].rearrange("p a b -> p (a b)"), ar_out[:])
        nc.vector.tensor_mul(
            pool[:], pool[:],
            rcnt[:].unsqueeze(1).broadcast_to([128, FCH, NG]))
        poolb = big.tile([128, FCH, NG], bf)
        nc.vector.tensor_copy(poolb[:], pool[:])

        # ---- MLP (transposed) ----
        mw1 = cst.tile([128, FCH, OC, 128], bf)
        nc.sync.dma_start(mw1[:].rearrange("p a b q -> p (a b q)"), mw1_d[:])
        mw2 = cst.tile([128, OC, 128], bf)
        nc.sync.dma_start(mw2[:].rearrange("p a b -> p (a b)"), mw2_d[:])
        hdd = big.tile([128, OC, NG], bf)
        for cc in range(OC):
            hps = pp.tile([128, NG], f32, tag="poolk")
            for k in range(FCH):
                nc.tensor.matmul(hps[:], lhsT=mw1[:, k, cc, :],
                                 rhs=poolb[:, k, :],
                                 start=(k == 0), stop=(k == FCH - 1))
            nc.vector.tensor_scalar(hdd[:, cc, :], hps[:],
                                    mb1[:, cc:cc + 1], 0.0, ALU.add, ALU.max)
        ops = pp.tile([128, NG], f32, tag="ops")
        for cc in range(OC):
            nc.tensor.matmul(ops[:], lhsT=mw2[:, cc, :], rhs=hdd[:, cc, :],
                             start=(cc == 0), stop=(cc == OC - 1))
        ofin = big.tile([128, NG], f32)
        nc.vector.tensor_scalar_add(ofin[:], ops[:], mb2[:, 0:1])
        nc.sync.dma_start(outT_d[:], ofin[:])

    nc.compile()
    return nc


# ---------------- host driver ----------------

def _prepare(x, edge_index, batch,
             conv1_W, conv1_asrc, conv1_adst, conv1_b,
             conv2_W, conv2_asrc, conv2_adst, conv2_b,
             mlp_W1, mlp_b1, mlp_W2, mlp_b2):
    c = CFG
    N, NPAD, IN_CH, OUT1, HEADS, HID = (c["N"], c["NPAD"], c["IN_CH"],
                                        c["OUT1"], c["HEADS"], c["HID"])
    NG, NCORE = c["N_GRAPHS"], c["NCORE"]
    NT = NPAD // 128
    TPC = NT // NCORE
    SLAB = TPC * 128
    pad_idx = N  # zero row

    src = np.asarray(edge_index[0], np.int64).astype(np.int32)
    dst = np.asarray(edge_index[1], np.int64).astype(np.int32)
    batch = np.asarray(batch, np.int64).astype(np.int32)
    loop = np.arange(N, dtype=np.int32)
    csrc = np.concatenate([src, loop])
    cdst = np.concatenate([dst, loop])

    deg = np.bincount(dst, minlength=N).astype(np.float32)
    dis = np.where(deg > 0, 1.0 / np.sqrt(np.maximum(deg, 1.0)), 0.0)
    wgt = dis[src] * dis[dst]

    EPG, gsp, gdp, gdl, _ = _edge_tiles(csrc, cdst, NT, pad_idx)
    EPL, lsp, _, ldl, lwp = _edge_tiles(src, dst, NT, pad_idx, wgt)

    # device row permutation for half-split AllGathers:
    # node n -> h*(NCORE*HALF) + core*HALF + i
    HALF = SLAB // 2

    def devrow(n):
        cc = n // SLAB
        l = n % SLAB
        return (l // HALF) * (NCORE * HALF) + cc * HALF + (l % HALF)

    gsp = devrow(gsp)
    gdp = devrow(gdp)
    lsp = devrow(lsp)
    Sg = _selector(gdl)
    Sl = _selector(ldl)

    # pooling one-hot
    bpad = np.full(NPAD, -1, np.int32)
    bpad[:N] = batch
    Spool = np.zeros((NT, 128, NG), np.uint8)
    ti = np.repeat(np.arange(NT), 128)
    pi = np.tile(np.arange(128), NT)
    v = bpad >= 0
    Spool[ti[v], pi[v], bpad[v]] = 1

    cnt = np.bincount(batch, minlength=NG).astype(np.float32)
    rcnt = (1.0 / np.maximum(cnt, 1.0)).astype(np.float32)

    x = np.asarray(x, np.float32)
    xp = np.zeros((NPAD, IN_CH), np.float32)
    xp[:N] = x
    xT = np.ascontiguousarray(xp.T).astype(BF16)
    xrows = xp.astype(BF16)

    def fold(W, asrc, adst):
        ws, wd = _fold_logit_w(np.asarray(W, np.float32),
                               np.asarray(asrc, np.float32),
                               np.asarray(adst, np.float32), HEADS, HID)
        return np.concatenate([np.asarray(W, np.float32), ws, wd],
                              axis=1).astype(BF16)

    w1e = fold(conv1_W, conv1_asrc, conv1_adst)
    w2e = fold(conv2_W, conv2_asrc, conv2_adst)
    b1r = np.tile(np.asarray(conv1_b, np.float32)[None, :],
                  (128, 1)).astype(BF16)
    b2r = np.tile(np.asarray(conv2_b, np.float32)[None, :],
                  (128, 1)).astype(BF16)

    W1 = np.asarray(mlp_W1, np.float32)     # [JK, 256]
    W2 = np.asarray(mlp_W2, np.float32)     # [256, 128]
    JK = W1.shape[0]
    FCH = JK // 128
    OC = W1.shape[1] // 128
    mw1 = np.ascontiguousarray(
        W1.reshape(FCH, 128, OC, 128).transpose(1, 0, 2, 3)
        .reshape(128, -1)).astype(BF16)
    mw2 = np.ascontiguousarray(
        W2.reshape(OC, 128, W2.shape[1]).transpose(1, 0, 2)
        .reshape(128, -1)).astype(BF16)
    mb1 = np.ascontiguousarray(
        np.asarray(mlp_b1, np.float32).reshape(OC, 128).T)
    mb2 = np.asarray(mlp_b2, np.float32).reshape(128, 1)
    rcT = np.tile(rcnt[None, :], (128, 1))

    in_maps = []
    for cix in range(NCORE):
        tl = slice(cix * TPC, (cix + 1) * TPC)
        rows = slice(cix * SLAB, (cix + 1) * SLAB)
        in_maps.append({
            "xts": np.ascontiguousarray(xT[:, rows]),
            "xr": np.ascontiguousarray(xrows[rows]),
            "w1e": w1e, "w2e": w2e, "b1r": b1r, "b2r": b2r,
            "gidx": _wrap_idx(gsp[tl]),
            "geidx": _wrap_idx(np.concatenate([gsp[tl], gdp[tl]], axis=1)),
            "sgat": _sel_layout(Sg[tl]),
            "lidx": _wrap_idx(lsp[tl]),
            "slp": _sel_layout(Sl[tl]),
            "wlp": _chunk_layout(lwp[tl]).astype(BF16),
            "spool": np.ascontiguousarray(
                Spool[tl].transpose(1, 0, 2).reshape(128, -1)).astype(F8),
            "mw1": mw1, "mw2": mw2, "mb1": mb1, "mb2": mb2, "rcnt": rcT,
        })
    return (EPG, EPL), in_maps


def kernel(x, edge_index, batch,
           conv1_W, conv1_asrc, conv1_adst, conv1_b,
           conv2_W, conv2_asrc, conv2_adst, conv2_b,
           mlp_W1, mlp_b1, mlp_W2, mlp_b2):
    NCORE = CFG["NCORE"]
    (EPG, EPL), in_maps = _prepare(
        x, edge_index, batch, conv1_W, conv1_asrc, conv1_adst, conv1_b,
        conv2_W, conv2_asrc, conv2_adst, conv2_b,
        mlp_W1, mlp_b1, mlp_W2, mlp_b2)
    key = (EPG, EPL, NCORE)
    if _cached.get("key") != key:
        _cached["nc"] = _build(key)
        _cached["key"] = key
    nc = _cached["nc"]

    if os.environ.get("BASS_KERNEL_SIM", "0") == "1":
        from concourse.bass_interp import MultiCoreSim
        nw = int(os.environ.get("BASS_SIM_WORKERS",
                                str(min(NCORE, os.cpu_count() or 1))))
        sim = MultiCoreSim(nc, num_cores=NCORE, require_finite=False,
                           num_workers=nw)
        for cix, cs in enumerate(sim.cores.values()):
            for k, v in in_maps[cix].items():
                cs.tensor(k)[:] = v
        sim.simulate()
        _cached["sim_time_ns"] = int(sim.global_time)
        outT = np.asarray(sim.cores[0].tensor("outT"))
    else:
        from concourse.bass_utils import run_bass_kernel_spmd
        import time
        t0 = time.time()
        res = run_bass_kernel_spmd(nc, in_maps, core_ids=list(range(NCORE)))
        _cached["device_wall_ns"] = int((time.time() - t0) * 1e9)
        _cached["last_result"] = res
        outT = res.results[0]["outT"]
    return np.ascontiguousarray(outT.T.astype(np.float32))


# revision 15
# speedup vs baseline: 2572.4418x; 1.1702x over previous
"""DSGIAT GraphBranch kernel for trn2: full-device implementation on 8 cores.

Pipeline (all on device, single launch):
  GEMM1 (row-sharded, attention logits folded as extra columns) -> AllGather
  -> GAT agg (dma_gather of src rows + one-hot selector matmuls; softmax
     without max-subtraction) -> AG -> LP x2 (gather + selector matmul) with
     AG between -> GEMM2 -> AG -> GAT2 -> AG -> LP x2 -> transposed pooling
     via one-hot batch matmul -> AllReduce -> replicated MLP (transposed).

Sharding: nodes row-sharded 8 ways (3840 rows/core of 30720 padded); each
sparse pass processes edges whose dst is in the core's slab; exchanges via
ncfw AllGather. Payloads bf16, selectors fp8 ({0,1} exact), accum fp32.
"""
import os
import numpy as np
import ml_dtypes
from contextlib import ExitStack

BF16 = ml_dtypes.bfloat16
F8 = ml_dtypes.float8_e4m3

# ---- sizes (full problem; test_sim overrides via set_config) ----
CFG = dict(
    N=30000, NPAD=30720, IN_CH=256, OUT1=512, HID=128, HEADS=4,
    N_GRAPHS=64, NCORE=8,
)

NEG_SLOPE = 0.2
EPS = 1e-16
PAD = None  # set from cfg: pad gather index (a real, all-zero row)

_cached = {}


def set_config(**kw):
    CFG.update(kw)
    _cached.clear()


# ---------------- host preprocessing ----------------

def _wrap_idx(idx2d):
    """[T, EP] int -> dma_gather layout [128, T*(EP//16)] int16."""
    T, EP = idx2d.shape
    a = idx2d.reshape(T, EP // 16, 16).astype(np.int16)
    w16 = a.transpose(2, 0, 1)                      # [16, T, S]
    w = np.tile(w16, (8, 1, 1))                     # [128, T, S]
    return np.ascontiguousarray(w.reshape(128, -1))


def _edge_tiles(src, dst, ntiles, pad_idx, wgt=None):
    """Sort edges by dst, pad per dst-tile of 128. Returns
    (EP, src_pad [T,EP], dst_pad [T,EP], dstl [T,EP], w_pad or None)."""
    order = np.argsort(dst, kind="stable")
    s, d = src[order], dst[order]
    t = d // 128
    cnt = np.bincount(t, minlength=ntiles)
    EP = max(128, int(-(-cnt.max() // 128)) * 128)
    offs = np.zeros(ntiles, np.int64)
    offs[1:] = np.cumsum(cnt)[:-1]
    pos = np.arange(len(s)) - offs[t]
    sp = np.full((ntiles, EP), pad_idx, np.int32)
    dp = np.full((ntiles, EP), pad_idx, np.int32)
    dl = np.full((ntiles, EP), -1, np.int32)
    sp[t, pos] = s
    dp[t, pos] = d
    dl[t, pos] = d - t * 128
    wp = None
    if wgt is not None:
        wp = np.zeros((ntiles, EP), np.float32)
        wp[t, pos] = wgt[order]
    return EP, sp, dp, dl, wp


def _selector(dl):
    """One-hot [T, EP, 128] fp8 from dst-local indices (-1 -> zero row)."""
    T, EP = dl.shape
    S = np.zeros((T, EP, 128), np.uint8)
    ti, ei = np.nonzero(dl >= 0)
    S[ti, ei, dl[ti, ei]] = 1
    return S


def _sel_layout(S):
    """[T, EP, 128] -> [128, T*EP] fp8 (edge e=c*128+p of tile t at
    [p, t*EP + c*128 + n])."""
    T, EP, _ = S.shape
    CH = EP // 128
    out = S.reshape(T, CH, 128, 128).transpose(2, 0, 1, 3).reshape(128, -1)
    return out.astype(F8)


def _chunk_layout(v):
    """[T, EP] -> [128, T*(EP//128)]: value of edge c*128+p at [p, t*CH+c]."""
    T, EP = v.shape
    CH = EP // 128
    return np.ascontiguousarray(
        v.reshape(T, CH, 128).transpose(2, 0, 1).reshape(128, -1))


def _fold_logit_w(W, a_src, a_dst, heads, hid):
    ws = np.stack([W[:, h * hid:(h + 1) * hid] @ a_src[h] for h in range(heads)],
                  axis=1)
    wd = np.stack([W[:, h * hid:(h + 1) * hid] @ a_dst[h] for h in range(heads)],
                  axis=1)
    return ws, wd  # [in, heads]


# ---------------- device program ----------------

def _build(key):
    import concourse.tile as tile
    from concourse import bacc, mybir, bass

    EPG, EPL, NCORE = key
    c = CFG
    NPAD, IN_CH, OUT1, HEADS = c["NPAD"], c["IN_CH"], c["OUT1"], c["HEADS"]
    NG = c["N_GRAPHS"]
    NT = NPAD // 128
    TPC = NT // NCORE
    SLAB = TPC * 128
    KIN = IN_CH // 128          # k-chunks for GEMM1
    KH = OUT1 // 128            # k-chunks for GEMM2 / feature blocks
    CHG = EPG // 128
    CHL = EPL // 128
    GS = EPG // 16              # idx slots per tile (gather1)
    LS = EPL // 16
    W1C = OUT1 + 2 * HEADS      # 520
    STR = ((W1C * 2 + 255) // 256) * 128  # row stride elems (640) bf16
    JK = IN_CH + 2 * OUT1
    FCH = JK // 128             # 10 pooled feature chunks
    MLP1 = 256
    OC = MLP1 // 128            # 2

    bf = mybir.dt.bfloat16
    f32 = mybir.dt.float32
    f8 = mybir.dt.float8e4
    i16 = mybir.dt.int16

    nc = bacc.Bacc("TRN2", target_bir_lowering=False, debug=False,
                   num_devices=NCORE)

    # ---- inputs ----
    xts_d = nc.dram_tensor("xts", [IN_CH, SLAB], bf, kind="ExternalInput")
    xr_d = nc.dram_tensor("xr", [SLAB, IN_CH], bf, kind="ExternalInput")
    w1_d = nc.dram_tensor("w1e", [IN_CH, W1C], bf, kind="ExternalInput")
    w2_d = nc.dram_tensor("w2e", [OUT1, W1C], bf, kind="ExternalInput")
    b1_d = nc.dram_tensor("b1r", [128, OUT1], bf, kind="ExternalInput")
    b2_d = nc.dram_tensor("b2r", [128, OUT1], bf, kind="ExternalInput")
    gidx_d = nc.dram_tensor("gidx", [128, TPC * GS], i16, kind="ExternalInput")
    geidx_d = nc.dram_tensor("geidx", [128, TPC * 2 * GS], i16,
                             kind="ExternalInput")
    sgat_d = nc.dram_tensor("sgat", [128, TPC * EPG], f8, kind="ExternalInput")
    lidx_d = nc.dram_tensor("lidx", [128, TPC * LS], i16, kind="ExternalInput")
    slp_d = nc.dram_tensor("slp", [128, TPC * EPL], f8, kind="ExternalInput")
    wlp_d = nc.dram_tensor("wlp", [128, TPC * CHL], bf, kind="ExternalInput")
    spool_d = nc.dram_tensor("spool", [128, TPC * NG], f8, kind="ExternalInput")
    mw1_d = nc.dram_tensor("mw1", [128, FCH * OC * 128], bf,
                           kind="ExternalInput")
    mw2_d = nc.dram_tensor("mw2", [128, OC * 128], bf, kind="ExternalInput")
    mb1_d = nc.dram_tensor("mb1", [128, OC], f32, kind="ExternalInput")
    mb2_d = nc.dram_tensor("mb2", [128, 1], f32, kind="ExternalInput")
    rcnt_d = nc.dram_tensor("rcnt", [128, NG], f32, kind="ExternalInput")
    outT_d = nc.dram_tensor("outT", [128, NG], f32, kind="ExternalOutput")

    # ---- internal DRAM ----
    def idram(name, shape, dt, shared=False):
        return nc.dram_tensor(name, shape, dt, kind="Internal",
                              addr_space="Shared" if shared else "Local")

    sh = NCORE > 4
    h1p_own = idram("h1p_own", [SLAB, OUT1], f8)
    h1p = idram("h1p_full", [NPAD, OUT1], f8, shared=sh)
    e1p_own = idram("e1p_own", [SLAB, 128], bf)
    e1p = idram("e1p_full", [NPAD, 128], bf, shared=sh)
    h10_own = idram("h10_own", [SLAB, OUT1], f8)
    h10 = idram("h10_full", [NPAD, OUT1], f8, shared=sh)
    h1a_own = idram("h1a_own", [SLAB, OUT1], f8)
    h1a = idram("h1a_full", [NPAD, OUT1], f8, shared=sh)
    h1f_own = idram("h1f_own", [SLAB, OUT1], bf)
    h2p_own = idram("h2p_own", [SLAB, OUT1], f8)
    h2p = idram("h2p_full", [NPAD, OUT1], f8, shared=sh)
    e2p_own = idram("e2p_own", [SLAB, 128], bf)
    e2p = idram("e2p_full", [NPAD, 128], bf, shared=sh)
    h20_own = idram("h20_own", [SLAB, OUT1], f8)
    h20 = idram("h20_full", [NPAD, OUT1], f8, shared=sh)
    h2a_own = idram("h2a_own", [SLAB, OUT1], f8)
    h2a = idram("h2a_full", [NPAD, OUT1], f8, shared=sh)
    h2f_own = idram("h2f_own", [SLAB, OUT1], bf)
    ar_in = idram("ar_in", [128, FCH * NG], f32)
    ar_out = idram("ar_out", [128, FCH * NG], f32, shared=sh)

    ts = bass.ts
    RG = [list(range(NCORE))]
    AF = mybir.ActivationFunctionType
    ALU = mybir.AluOpType

    with tile.TileContext(nc) as tc, ExitStack() as ctx:
        cst = ctx.enter_context(tc.tile_pool(name="cst", bufs=1))
        big = ctx.enter_context(tc.tile_pool(name="big", bufs=1))
        sb = ctx.enter_context(tc.tile_pool(name="sb", bufs=2))
        ps = ctx.enter_context(tc.tile_pool(name="ps", bufs=2, space="PSUM"))
        pp = ctx.enter_context(tc.tile_pool(name="pp", bufs=2, space="PSUM"))

        # resident constants
        xts = big.tile([128, KIN, SLAB], bf, tag="glhs")
        for k in range(KIN):
            nc.sync.dma_start(xts[:, k, :], xts_d[ts(k, 128), :])
        w1 = cst.tile([128, KIN, W1C], bf)
        for k in range(KIN):
            nc.sync.dma_start(w1[:, k, :], w1_d[ts(k, 128), :])
        w2 = cst.tile([128, KH, W1C], bf)
        for k in range(KH):
            nc.sync.dma_start(w2[:, k, :], w2_d[ts(k, 128), :])
        b1r = cst.tile([128, OUT1], bf)
        nc.sync.dma_start(b1r[:], b1_d[:])
        b2r = cst.tile([128, OUT1], bf)
        nc.sync.dma_start(b2r[:], b2_d[:])
        gidx = cst.tile([128, TPC * GS], i16)
        nc.sync.dma_start(gidx[:], gidx_d[:])
        geidx = cst.tile([128, TPC * 2 * GS], i16)
        nc.sync.dma_start(geidx[:], geidx_d[:])
        lidx = cst.tile([128, TPC * LS], i16)
        nc.sync.dma_start(lidx[:], lidx_d[:])
        wlp = cst.tile([128, TPC * CHL], bf)
        nc.sync.dma_start(wlp[:], wlp_d[:])
        spool = cst.tile([128, TPC * NG], f8)
        nc.sync.dma_start(spool[:], spool_d[:])
        rcnt = cst.tile([128, NG], f32)
        nc.sync.dma_start(rcnt[:], rcnt_d[:])
        mb1 = cst.tile([128, OC], f32)
        nc.sync.dma_start(mb1[:], mb1_d[:])
        mb2 = cst.tile([128, 1], f32)
        nc.sync.dma_start(mb2[:], mb2_d[:])

        def gemm(wt, kch, src_get, dst, edst):
            """dst[t rows] = lhsT_chunks^T @ wt ([128,kch,W1C])."""
            for t in range(TPC):
                pa = ps.tile([128, OUT1], f32, tag="acc")
                pb = ps.tile([128, 2 * HEADS], f32, tag="acc2")
                for k in range(kch):
                    lh = src_get(k, t)
                    nc.tensor.matmul(pa[:], lhsT=lh, rhs=wt[:, k, 0:OUT1],
                                     start=(k == 0), stop=(k == kch - 1))
                    nc.tensor.matmul(pb[:], lhsT=lh, rhs=wt[:, k, OUT1:W1C],
                                     start=(k == 0), stop=(k == kch - 1))
                ot = sb.tile([128, OUT1], f8, tag="geo")
                nc.vector.tensor_copy(ot[:], pa[:])
                nc.sync.dma_start(dst[ts(t, 128), :], ot[:])
                oe = sb.tile([128, 128], bf, tag="geoe")
                nc.vector.memset(oe[:, 2 * HEADS:128], 0.0)
                nc.vector.tensor_copy(oe[:, 0:2 * HEADS], pb[:])
                nc.sync.dma_start(edst[ts(t, 128), :], oe[:])

        # ---- GEMM1 ----
        gemm(w1, KIN, lambda k, t: xts[:, k, ts(t, 128)], h1p_own, e1p_own)
        nc.gpsimd.collective_compute("AllGather", ALU.bypass,
                                     replica_groups=RG,
                                     ins=[h1p_own[:]], outs=[h1p[:]])
        nc.gpsimd.collective_compute("AllGather", ALU.bypass,
                                     replica_groups=RG,
                                     ins=[e1p_own[:]], outs=[e1p[:]])

        def gat_pass(hp_full, ep_full, out_own, out_bounce):
            for t in range(TPC):
                gh = sb.tile([128, CHG, OUT1], f8, tag="gbig")
                nc.gpsimd.dma_gather(
                    out_ap=gh[:], in_ap=hp_full[:],
                    idxs_ap=gidx[:, ts(t, GS)], num_idxs=EPG,
                    num_idxs_reg=EPG, elem_size=OUT1, elem_step=OUT1,
                    single_packet=False)
                ge = sb.tile([128, 2 * CHG, 128], bf, tag="ge")
                nc.gpsimd.dma_gather(
                    out_ap=ge[:], in_ap=ep_full[:],
                    idxs_ap=geidx[:, ts(t, 2 * GS)], num_idxs=2 * EPG,
                    num_idxs_reg=2 * EPG, elem_size=128,
                    elem_step=128, single_packet=False)
                st = sb.tile([128, CHG, 128], f8, tag="sel")
                nc.sync.dma_start(
                    st[:].rearrange("p a b -> p (a b)"), sgat_d[:, ts(t, EPG)])
                lg = sb.tile([128, CHG, HEADS], f32, tag="lg")
                nc.vector.tensor_add(lg[:], ge[:, 0:CHG, 0:HEADS],
                                     ge[:, CHG:2 * CHG, HEADS:2 * HEADS])
                # leaky_relu: max(0.2*x, x) in one DVE op
                nc.vector.scalar_tensor_tensor(
                    lg[:], lg[:], NEG_SLOPE, lg[:], ALU.mult, ALU.max)
                ex = sb.tile([128, CHG, HEADS], bf, tag="ex")
                nc.scalar.activation(ex[:], lg[:], AF.Exp)
                msg = sb.tile([128, CHG, OUT1], bf, tag="lmsg")
                nc.vector.tensor_mul(
                    msg[:].rearrange("p a (h q) -> p a h q", h=HEADS),
                    gh[:].rearrange("p a (h q) -> p a h q", h=HEADS),
                    ex[:].unsqueeze(3).broadcast_to(
                        [128, CHG, HEADS, OUT1 // HEADS]))
                pnum = ps.tile([128, OUT1], f32, tag="acc")
                pden = ps.tile([128, HEADS], f32, tag="acc2")
                for ch in range(CHG):
                    nc.tensor.matmul(pnum[:], lhsT=st[:, ch, :],
                                     rhs=msg[:, ch, :],
                                     start=(ch == 0), stop=(ch == CHG - 1))
                    nc.tensor.matmul(pden[:], lhsT=st[:, ch, :],
                                     rhs=ex[:, ch, :],
                                     start=(ch == 0), stop=(ch == CHG - 1))
                de = sb.tile([128, HEADS], f32, tag="de")
                nc.vector.tensor_scalar_add(de[:], pden[:], EPS)
                nc.vector.reciprocal(de[:], de[:])
                tmp = sb.tile([128, HEADS, OUT1 // HEADS], f32, tag="tf32")
                nc.vector.tensor_mul(
                    tmp[:], pnum[:].rearrange("p (h q) -> p h q", h=HEADS),
                    de[:].unsqueeze(2).broadcast_to(
                        [128, HEADS, OUT1 // HEADS]))
                ob = sb.tile([128, OUT1], f8, tag="obuf")
                nc.vector.tensor_add(
                    ob[:], tmp[:].rearrange("p h q -> p (h q)"), b1r[:])
                nc.vector.tensor_scalar_max(ob[:], ob[:], 0.0)
                nc.sync.dma_start(out_own[ts(t, 128), :], ob[:])
                if out_bounce is not None:
                    nc.sync.dma_start(out_bounce[ts(t, 128), :], ob[:])

        def lp_pass(h_full, res_own, out_own, out_dt):
            for t in range(TPC):
                g = sb.tile([128, CHL, OUT1], f8, tag="gbig")
                nc.gpsimd.dma_gather(
                    out_ap=g[:], in_ap=h_full[:], idxs_ap=lidx[:, ts(t, LS)],
                    num_idxs=EPL, num_idxs_reg=EPL, elem_size=OUT1,
                    elem_step=OUT1, single_packet=False)
                st = sb.tile([128, CHL, 128], f8, tag="sel")
                nc.sync.dma_start(
                    st[:].rearrange("p a b -> p (a b)"), slp_d[:, ts(t, EPL)])
                msg = sb.tile([128, CHL, OUT1], bf, tag="lmsg")
                nc.vector.tensor_mul(
                    msg[:], g[:],
                    wlp[:, ts(t, CHL)].unsqueeze(2).broadcast_to(
                        [128, CHL, OUT1]))
                pa = ps.tile([128, OUT1], f32, tag="acc")
                for ch in range(CHL):
                    nc.tensor.matmul(pa[:], lhsT=st[:, ch, :],
                                     rhs=msg[:, ch, :],
                                     start=(ch == 0), stop=(ch == CHL - 1))
                res = sb.tile([128, OUT1], f8, tag="lres")
                nc.sync.dma_start(res[:], res_own[ts(t, 128), :])
                tf = sb.tile([128, OUT1], f32, tag="tf32")
                nc.vector.tensor_add(tf[:], pa[:], res[:])
                ob = sb.tile([128, OUT1], out_dt, tag="obuf")
                nc.vector.tensor_scalar(ob[:], tf[:], 0.5, 1.0,
                                        ALU.mult, ALU.min)
                nc.vector.tensor_scalar_max(ob[:], ob[:], 0.0)
                nc.sync.dma_start(out_own[ts(t, 128), :], ob[:])

        # ---- conv1: GAT + LP x2 ----
        gat_pass(h1p, e1p, h10_own, None)
        nc.gpsimd.collective_compute("AllGather", ALU.bypass,
                                     replica_groups=RG,
                                     ins=[h10_own[:]], outs=[h10[:]])
        lp_pass(h10, h10_own, h1a_own, f8)
        nc.gpsimd.collective_compute("AllGather", ALU.bypass,
                                     replica_groups=RG,
                                     ins=[h1a_own[:]], outs=[h1a[:]])
        lp_pass(h1a, h10_own, h1f_own, bf)

        # ---- GEMM2 (lhsT via DMA transpose of h1f_own) ----
        h1t = big.tile([128, KH, SLAB], bf, tag="glhs")
        for k in range(KH):
            nc.sync.dma_start(h1t[:, k, :], h1f_own[:, ts(k, 128)],
                              transpose=True)
        gemm(w2, KH, lambda k, t: h1t[:, k, ts(t, 128)], h2p_own, e2p_own)
        nc.gpsimd.collective_compute("AllGather", ALU.bypass,
                                     replica_groups=RG,
                                     ins=[h2p_own[:]], outs=[h2p[:]])
        nc.gpsimd.collective_compute("AllGather", ALU.bypass,
                                     replica_groups=RG,
                                     ins=[e2p_own[:]], outs=[e2p[:]])

        # ---- conv2: GAT + LP x2 ----
        gat_pass(h2p, e2p, h20_own, None)
        nc.gpsimd.collective_compute("AllGather", ALU.bypass,
                                     replica_groups=RG,
                                     ins=[h20_own[:]], outs=[h20[:]])
        lp_pass(h20, h20_own, h2a_own, f8)
        nc.gpsimd.collective_compute("AllGather", ALU.bypass,
                                     replica_groups=RG,
                                     ins=[h2a_own[:]], outs=[h2a[:]])
        lp_pass(h2a, h20_own, h2f_own, bf)

        # ---- pooling (transposed): pooledT[f, g], one PSUM group per chunk --
        par = big.tile([128, FCH * NG], f32)
        psrc = ([(xr_d, k) for k in range(KIN)]
                + [(h1f_own, k) for k in range(KH)]
                + [(h2f_own, k) for k in range(KH)])
        for kk, (srcd, k) in enumerate(psrc):
            pk = pp.tile([128, NG], f32, tag="poolk")
            for t in range(TPC):
                lh = sb.tile([128, 128], bf, tag="plh")
                nc.sync.dma_start(lh[:], srcd[ts(t, 128), ts(k, 128)])
                nc.tensor.matmul(pk[:], lhsT=lh[:], rhs=spool[:, ts(t, NG)],
                                 start=(t == 0), stop=(t == TPC - 1))
            nc.vector.tensor_copy(par[:, ts(kk, NG)], pk[:])
        nc.sync.dma_start(ar_in[:], par[:])
        nc.gpsimd.collective_compute("AllReduce", ALU.add,
                                     replica_groups=RG,
                                     ins=[ar_in[:]], outs=[ar_out[:]])
        pool = big.tile([128, FCH, NG], f32)
        nc.sync.dma_start(pool[:].rearrange("p a b -> p (a b)"), ar_out[:])
        nc.vector.tensor_mul(
            pool[:], pool[:],
            rcnt[:].unsqueeze(1).broadcast_to([128, FCH, NG]))
        poolb = big.tile([128, FCH, NG], bf)
        nc.vector.tensor_copy(poolb[:], pool[:])

        # ---- MLP (transposed) ----
        mw1 = cst.tile([128, FCH, OC, 128], bf)
        nc.sync.dma_start(mw1[:].rearrange("p a b q -> p (a b q)"), mw1_d[:])
        mw2 = cst.tile([128, OC, 128], bf)
        nc.sync.dma_start(mw2[:].rearrange("p a b -> p (a b)"), mw2_d[:])
        hdd = big.tile([128, OC, NG], bf)
        for cc in range(OC):
            hps = pp.tile([128, NG], f32, tag="poolk")
            for k in range(FCH):
                nc.tensor.matmul(hps[:], lhsT=mw1[:, k, cc, :],
                                 rhs=poolb[:, k, :],
                                 start=(k == 0), stop=(k == FCH - 1))
            nc.vector.tensor_scalar(hdd[:, cc, :], hps[:],
                                    mb1[:, cc:cc + 1], 0.0, ALU.add, ALU.max)
        ops = pp.tile([128, NG], f32, tag="ops")
        for cc in range(OC):
            nc.tensor.matmul(ops[:], lhsT=mw2[:, cc, :], rhs=hdd[:, cc, :],
                             start=(cc == 0), stop=(cc == OC - 1))
        ofin = big.tile([128, NG], f32)
        nc.vector.tensor_scalar_add(ofin[:], ops[:], mb2[:, 0:1])
        nc.sync.dma_start(outT_d[:], ofin[:])

    nc.compile()
    return nc


# ---------------- host driver ----------------

def _prepare(x, edge_index, batch,
             conv1_W, conv1_asrc, conv1_adst, conv1_b,
             conv2_W, conv2_asrc, conv2_adst, conv2_b,
             mlp_W1, mlp_b1, mlp_W2, mlp_b2):
    c = CFG
    N, NPAD, IN_CH, OUT1, HEADS, HID = (c["N"], c["NPAD"], c["IN_CH"],
                                        c["OUT1"], c["HEADS"], c["HID"])
    NG, NCORE = c["N_GRAPHS"], c["NCORE"]
    NT = NPAD // 128
    TPC = NT // NCORE
    SLAB = TPC * 128
    pad_idx = N  # zero row

    src = np.asarray(edge_index[0], np.int64).astype(np.int32)
    dst = np.asarray(edge_index[1], np.int64).astype(np.int32)
    batch = np.asarray(batch, np.int64).astype(np.int32)
    loop = np.arange(N, dtype=np.int32)
    csrc = np.concatenate([src, loop])
    cdst = np.concatenate([dst, loop])

    deg = np.bincount(dst, minlength=N).astype(np.float32)
    dis = np.where(deg > 0, 1.0 / np.sqrt(np.maximum(deg, 1.0)), 0.0)
    wgt = dis[src] * dis[dst]

    EPG, gsp, gdp, gdl, _ = _edge_tiles(csrc, cdst, NT, pad_idx)
    EPL, lsp, _, ldl, lwp = _edge_tiles(src, dst, NT, pad_idx, wgt)
    Sg = _selector(gdl)
    Sl = _selector(ldl)

    # pooling one-hot
    bpad = np.full(NPAD, -1, np.int32)
    bpad[:N] = batch
    Spool = np.zeros((NT, 128, NG), np.uint8)
    ti = np.repeat(np.arange(NT), 128)
    pi = np.tile(np.arange(128), NT)
    v = bpad >= 0
    Spool[ti[v], pi[v], bpad[v]] = 1

    cnt = np.bincount(batch, minlength=NG).astype(np.float32)
    rcnt = (1.0 / np.maximum(cnt, 1.0)).astype(np.float32)

    x = np.asarray(x, np.float32)
    xp = np.zeros((NPAD, IN_CH), np.float32)
    xp[:N] = x
    xT = np.ascontiguousarray(xp.T).astype(BF16)
    xrows = xp.astype(BF16)

    def fold(W, asrc, adst):
        ws, wd = _fold_logit_w(np.asarray(W, np.float32),
                               np.asarray(asrc, np.float32),
                               np.asarray(adst, np.float32), HEADS, HID)
        return np.concatenate([np.asarray(W, np.float32), ws, wd],
                              axis=1).astype(BF16)

    w1e = fold(conv1_W, conv1_asrc, conv1_adst)
    w2e = fold(conv2_W, conv2_asrc, conv2_adst)
    b1r = np.tile(np.asarray(conv1_b, np.float32)[None, :],
                  (128, 1)).astype(BF16)
    b2r = np.tile(np.asarray(conv2_b, np.float32)[None, :],
                  (128, 1)).astype(BF16)

    W1 = np.asarray(mlp_W1, np.float32)     # [JK, 256]
    W2 = np.asarray(mlp_W2, np.float32)     # [256, 128]
    JK = W1.shape[0]
    FCH = JK // 128
    OC = W1.shape[1] // 128
    mw1 = np.ascontiguousarray(
        W1.reshape(FCH, 128, OC, 128).transpose(1, 0, 2, 3)
        .reshape(128, -1)).astype(BF16)
    mw2 = np.ascontiguousarray(
        W2.reshape(OC, 128, W2.shape[1]).transpose(1, 0, 2)
        .reshape(128, -1)).astype(BF16)
    mb1 = np.ascontiguousarray(
        np.asarray(mlp_b1, np.float32).reshape(OC, 128).T)
    mb2 = np.asarray(mlp_b2, np.float32).reshape(128, 1)
    rcT = np.tile(rcnt[None, :], (128, 1))

    in_maps = []
    for cix in range(NCORE):
        tl = slice(cix * TPC, (cix + 1) * TPC)
        rows = slice(cix * SLAB, (cix + 1) * SLAB)
        in_maps.append({
            "xts": np.ascontiguousarray(xT[:, rows]),
            "xr": np.ascontiguousarray(xrows[rows]),
            "w1e": w1e, "w2e": w2e, "b1r": b1r, "b2r": b2r,
            "gidx": _wrap_idx(gsp[tl]),
            "geidx": _wrap_idx(np.concatenate([gsp[tl], gdp[tl]], axis=1)),
            "sgat": _sel_layout(Sg[tl]),
            "lidx": _wrap_idx(lsp[tl]),
            "slp": _sel_layout(Sl[tl]),
            "wlp": _chunk_layout(lwp[tl]).astype(BF16),
            "spool": np.ascontiguousarray(
                Spool[tl].transpose(1, 0, 2).reshape(128, -1)).astype(F8),
            "mw1": mw1, "mw2": mw2, "mb1": mb1, "mb2": mb2, "rcnt": rcT,
        })
    return (EPG, EPL), in_maps


def kernel(x, edge_index, batch,
           conv1_W, conv1_asrc, conv1_adst, conv1_b,
           conv2_W, conv2_asrc, conv2_adst, conv2_b,
           mlp_W1, mlp_b1, mlp_W2, mlp_b2):
    NCORE = CFG["NCORE"]
    (EPG, EPL), in_maps = _prepare(
        x, edge_index, batch, conv1_W, conv1_asrc, conv1_adst, conv1_b,
        conv2_W, conv2_asrc, conv2_adst, conv2_b,
        mlp_W1, mlp_b1, mlp_W2, mlp_b2)
    key = (EPG, EPL, NCORE)
    if _cached.get("key") != key:
        _cached["nc"] = _build(key)
        _cached["key"] = key
    nc = _cached["nc"]

    if os.environ.get("BASS_KERNEL_SIM", "0") == "1":
        from concourse.bass_interp import MultiCoreSim
        nw = int(os.environ.get("BASS_SIM_WORKERS",
                                str(min(NCORE, os.cpu_count() or 1))))
        sim = MultiCoreSim(nc, num_cores=NCORE, require_finite=False,
                           num_workers=nw)
        for cix, cs in enumerate(sim.cores.values()):
            for k, v in in_maps[cix].items():
                cs.tensor(k)[:] = v
        sim.simulate()
        _cached["sim_time_ns"] = int(sim.global_time)
        outT = np.asarray(sim.cores[0].tensor("outT"))
    else:
        from concourse.bass_utils import run_bass_kernel_spmd
        import time
        t0 = time.time()
        res = run_bass_kernel_spmd(nc, in_maps, core_ids=list(range(NCORE)))
        _cached["device_wall_ns"] = int((time.time() - t0) * 1e9)
        _cached["last_result"] = res
        outT = res.results[0]["outT"]
    return np.ascontiguousarray(outT.T.astype(np.float32))
